# revision 3
# baseline (speedup 1.0000x reference)
"""Trainium2 Bass kernel for LocalDualDirectedMessagePassingLayer.

Strategy (8 cores, dest-sharded):
  - Each core owns 1024 destination segments (8 blocks of 128 dests).
  - dest_seg is sorted, so each dest block's edges are contiguous; host pads
    each block's edge list to NT_B*128 and packs per core:
      srcTi [128, 2*e_cap]  chunk-interleaved (mem|feat per chunk) bf16
      efts  [97, e_cap]     concat(edge_features[edge_ids], time_enc, ones).T
      ldest [128, NT]       per-tile local-dest column (-1 for padding)
      scale [128, NT]       per-tile 1/cnt[dest] column
  - Device: one-hot S tiles are built on GpSimd from an iota constant
    (is_equal vs ldest, fused *scale), so no one-hot matrix is DMAed.
    Per 4-tile sup: read MLP (2 matmuls, N=512) + relu; per tile msg MLP
    (2 matmuls N=128); one DVE relu per sup; agg matmuls software-pipelined
    one sup behind so the PE never stalls on the DVE.
  - Per block: dst-side MLP chain (agg/upd/write) -> tanh -> writeT.
  - Host: transpose writeT, scatter rows into a copy of node_memory.
All matmul operands bf16, PSUM accumulation fp32.
"""

import sys

sys.path.insert(0, "/opt/trn_rl_repo")

import math

import ml_dtypes
import numpy as np

import concourse.bass as bass
import concourse.mybir as mybir
import concourse.tile as tile
from concourse import bacc
from concourse.bass_utils import run_bass_kernel_spmd

BF16 = ml_dtypes.bfloat16
N_CORES = 8
P = 128
N_DEST = 8192
D_MEM = 128

_PROG_CACHE: dict[int, object] = {}


def _chunk_plan(nt_b: int, first_block: bool):
    """Split nt_b tiles into DMA chunks (multiples of 4 tiles, <=16, plus a
    tail chunk equal to nt_b%4). Block 0 splits its first chunk small so the
    PE starts early."""
    tail = nt_b % 4
    body = nt_b - tail
    chunks = []
    while body > 0:
        c = min(16, body)
        chunks.append(c)
        body -= c
    if tail:
        chunks.append(tail)
    if first_block and chunks and chunks[0] >= 8:
        chunks = [4, chunks[0] - 4] + chunks[1:]
    return chunks


def _sup_plan(nt_b: int):
    """4-tile super-tiles, with one tail sup of nt_b%4 tiles."""
    sups = [4] * (nt_b // 4)
    if nt_b % 4:
        sups.append(nt_b % 4)
    return sups


def _build_program(nt_b: int):
    NT = 8 * nt_b
    e_cap = NT * P

    nc = bacc.Bacc("TRN2", target_bir_lowering=False, debug=False,
                   num_devices=N_CORES)
    f32 = mybir.dt.float32
    bf16 = mybir.dt.bfloat16
    AF = mybir.ActivationFunctionType
    OP = mybir.AluOpType

    srcTi = nc.dram_tensor("srcTi", [P, 2 * e_cap], bf16, kind="ExternalInput")
    efts = nc.dram_tensor("efts", [97, e_cap], bf16, kind="ExternalInput")
    ldest = nc.dram_tensor("ldest", [P, NT], f32, kind="ExternalInput")
    scale = nc.dram_tensor("scale", [P, NT], f32, kind="ExternalInput")
    iota = nc.dram_tensor("iota", [P, 512], bf16, kind="ExternalInput")
    dstT = nc.dram_tensor("dstT", [2, P, 1024], bf16, kind="ExternalInput")
    wr = nc.dram_tensor("wr", [2, P, P], bf16, kind="ExternalInput")
    wm0 = nc.dram_tensor("wm0", [P, P], bf16, kind="ExternalInput")
    wm1 = nc.dram_tensor("wm1", [97, P], bf16, kind="ExternalInput")
    wa = nc.dram_tensor("wa", [2, P, P], bf16, kind="ExternalInput")
    wu = nc.dram_tensor("wu", [2, P, P], bf16, kind="ExternalInput")
    ww = nc.dram_tensor("ww", [P, P], bf16, kind="ExternalInput")
    br = nc.dram_tensor("br", [P, 1], f32, kind="ExternalInput")
    ba = nc.dram_tensor("ba", [P, 1], f32, kind="ExternalInput")
    bu = nc.dram_tensor("bu", [P, 1], f32, kind="ExternalInput")
    bw = nc.dram_tensor("bw", [P, 1], f32, kind="ExternalInput")
    out_d = nc.dram_tensor("writeT", [P, 1024], f32, kind="ExternalOutput")

    with tile.TileContext(nc) as tc:
        with (
            tc.tile_pool(name="const", bufs=1) as cp,
            tc.tile_pool(name="io", bufs=4) as iop,
            tc.tile_pool(name="mid", bufs=8) as midp,
            tc.tile_pool(name="sp", bufs=6) as spool,
            tc.tile_pool(name="rdps", bufs=2, space="PSUM") as rdps,
            tc.tile_pool(name="mgps", bufs=2, space="PSUM") as mgps,
            tc.tile_pool(name="aggps", bufs=2, space="PSUM") as aggps,
            tc.tile_pool(name="dstps", bufs=1, space="PSUM") as dstps,
        ):
            def cload(ap, shape, dtype, tag):
                t = cp.tile(shape, dtype, tag=tag)
                nc.scalar.dma_start(out=t[:], in_=ap)
                return t

            # critical-path constants first (PE read weights, gpsimd S inputs)
            wr0 = cload(wr[0, :, :], [P, P], bf16, "wr0")
            wr1 = cload(wr[1, :, :], [P, P], bf16, "wr1")
            br_t = cload(br[:, :], [P, 1], f32, "br")
            iota_t = cload(iota[:, :], [P, 512], bf16, "iota")
            ld_t = cload(ldest[:, :], [P, NT], f32, "ldest")
            sc_t = cload(scale[:, :], [P, NT], f32, "scale")
            wm0_t = cload(wm0[:, :], [P, P], bf16, "wm0")
            wm1_t = cload(wm1[:, :], [97, P], bf16, "wm1")
            dstT0 = cload(dstT[0, :, :], [P, 1024], bf16, "dstT0")
            dstT1 = cload(dstT[1, :, :], [P, 1024], bf16, "dstT1")
            wa0 = cload(wa[0, :, :], [P, P], bf16, "wa0")
            wa1 = cload(wa[1, :, :], [P, P], bf16, "wa1")
            wu0 = cload(wu[0, :, :], [P, P], bf16, "wu0")
            wu1 = cload(wu[1, :, :], [P, P], bf16, "wu1")
            ww_t = cload(ww[:, :], [P, P], bf16, "ww")
            ba_t = cload(ba[:, :], [P, 1], f32, "ba")
            bu_t = cload(bu[:, :], [P, 1], f32, "bu")
            bw_t = cload(bw[:, :], [P, 1], f32, "bw")

            def dst_stage(b, agg_ps, stage, hold):
                dc = slice(b * P, (b + 1) * P)
                if stage == 0:
                    mmean = midp.tile([P, P], bf16, tag="mmean")
                    nc.vector.tensor_copy(mmean[:], agg_ps[:])
                    drp = dstps.tile([P, P], f32, tag="dst")
                    nc.tensor.matmul(drp[:], lhsT=wr0[:], rhs=dstT0[:, dc],
                                     start=True, stop=False)
                    nc.tensor.matmul(drp[:], lhsT=wr1[:], rhs=dstT1[:, dc],
                                     start=False, stop=True)
                    dstr = midp.tile([P, P], bf16, tag="dstr")
                    nc.scalar.activation(dstr[:], drp[:], AF.Relu, bias=br_t[:, :1])
                    hold.update(mmean=mmean, dstr=dstr)
                elif stage == 1:
                    agp = dstps.tile([P, P], f32, tag="dst")
                    nc.tensor.matmul(agp[:], lhsT=wa0[:], rhs=hold["dstr"][:],
                                     start=True, stop=False)
                    nc.tensor.matmul(agp[:], lhsT=wa1[:], rhs=hold["mmean"][:],
                                     start=False, stop=True)
                    aggT = midp.tile([P, P], bf16, tag="aggT")
                    nc.scalar.activation(aggT[:], agp[:], AF.Relu, bias=ba_t[:, :1])
                    hold.update(aggT=aggT)
                elif stage == 2:
                    upp = dstps.tile([P, P], f32, tag="dst")
                    nc.tensor.matmul(upp[:], lhsT=wu0[:], rhs=hold["aggT"][:],
                                     start=True, stop=False)
                    nc.tensor.matmul(upp[:], lhsT=wu1[:], rhs=hold["dstr"][:],
                                     start=False, stop=True)
                    updT = midp.tile([P, P], bf16, tag="updT")
                    nc.scalar.activation(updT[:], upp[:], AF.Relu, bias=bu_t[:, :1])
                    hold.update(updT=updT)
                else:
                    wrp = dstps.tile([P, P], f32, tag="dst")
                    nc.tensor.matmul(wrp[:], lhsT=ww_t[:], rhs=hold["updT"][:],
                                     start=True, stop=True)
                    wout = midp.tile([P, P], f32, tag="wout")
                    nc.scalar.activation(wout[:], wrp[:], AF.Tanh, bias=bw_t[:, :1])
                    nc.sync.dma_start(out=out_d[:, dc], in_=wout[:])

            sups = _sup_plan(nt_b)
            n_sup_b = len(sups)

            # global sup list: (block, tile0_global, ntiles, new_chunk_info)
            sup_list = []
            for b in range(8):
                chunks = _chunk_plan(nt_b, b == 0)
                # chunk start offsets in tiles (within block)
                coff, chunk_bounds = 0, []
                for csz in chunks:
                    chunk_bounds.append((coff, csz))
                    coff += csz
                soff = 0
                ci = 0
                for ntile in sups:
                    newc = None
                    if ci < len(chunk_bounds) and chunk_bounds[ci][0] == soff:
                        newc = chunk_bounds[ci]
                        ci += 1
                    sup_list.append((b, b * nt_b + soff, ntile, newc))
                    soff += ntile

            prev = None          # info for sup s-1 awaiting agg matmuls
            pending = None       # (block, agg_ps) awaiting dst stages
            hold = {}
            dst_ct = 4
            agg_ps = aggps.tile([P, P], f32, tag="agg")

            def emit_agg(pv):
                nonlocal pending, agg_ps, dst_ct, hold
                (pb, ptile0, pnt, pmsgs, pS4) = pv
                t_in_b = ptile0 - pb * nt_b
                for q in range(pnt):
                    qs = slice(q * P, (q + 1) * P)
                    nc.tensor.matmul(agg_ps[:], lhsT=pmsgs[:, qs],
                                     rhs=pS4[:, qs],
                                     start=(t_in_b + q == 0),
                                     stop=(t_in_b + q == nt_b - 1))
                if t_in_b + pnt == nt_b:
                    pending = (pb, agg_ps)
                    dst_ct = 0
                    hold = {}
                    agg_ps = aggps.tile([P, P], f32, tag="agg")

            cur_chunk = None  # (ct, ef, w) current chunk tiles
            for s, (b, tile0, ntile, newc) in enumerate(sup_list):
                if newc is not None:
                    c_t0, c_sz = newc
                    w = c_sz * P
                    o = (b * nt_b + c_t0) * P
                    ct = iop.tile([P, 2 * 16 * P], bf16, tag="srcT")
                    nc.sync.dma_start(out=ct[:, :2 * w],
                                      in_=srcTi[:, 2 * o:2 * o + 2 * w])
                    ef = iop.tile([97, 16 * P], bf16, tag="ef")
                    nc.sync.dma_start(out=ef[:, :w], in_=efts[:, o:o + w])
                    cur_chunk = (ct, ef, w, c_t0)

                ct, ef, w, c_t0 = cur_chunk
                j = (tile0 - b * nt_b - c_t0) * P   # col offset within chunk
                W = ntile * P
                s0 = ct[:, j:j + W]
                s1 = ct[:, w + j:w + j + W]
                efs = ef[:, j:j + W]

                # S tiles for this sup (gpsimd, depends only on constants)
                S4 = spool.tile([P, 512], bf16, tag="S4")
                for q in range(ntile):
                    t_glob = tile0 + q
                    nc.gpsimd.tensor_scalar(
                        S4[:, q * P:(q + 1) * P], iota_t[:, q * P:(q + 1) * P],
                        ld_t[:, t_glob:t_glob + 1], sc_t[:, t_glob:t_glob + 1],
                        OP.is_equal, OP.mult)

                # read MLP
                rd = rdps.tile([P, 512], f32, tag="rd")
                nc.tensor.matmul(rd[:, :W], lhsT=wr0[:], rhs=s0,
                                 start=True, stop=False)
                nc.tensor.matmul(rd[:, :W], lhsT=wr1[:], rhs=s1,
                                 start=False, stop=True)
                srT = midp.tile([P, 512], bf16, tag="srT")
                nc.scalar.activation(srT[:, :W], rd[:, :W], AF.Relu,
                                     bias=br_t[:, :1])

                # msg MLP per tile
                mg4 = mgps.tile([P, 512], f32, tag="mg")
                for q in range(ntile):
                    qs = slice(q * P, (q + 1) * P)
                    nc.tensor.matmul(mg4[:, qs], lhsT=srT[:, qs], rhs=wm0_t[:],
                                     start=True, stop=False)
                    nc.tensor.matmul(mg4[:, qs], lhsT=efs[:, qs], rhs=wm1_t[:],
                                     start=False, stop=True)

                # agg matmuls for previous sup (msgs ready by now)
                if prev is not None:
                    emit_agg(prev)

                msgs4 = midp.tile([P, 512], bf16, tag="msgs")
                nc.vector.tensor_scalar_max(msgs4[:, :W], mg4[:, :W], 0.0)
                prev = (b, tile0, ntile, msgs4, S4)

                if pending is not None and dst_ct < 4:
                    dst_stage(pending[0], pending[1], dst_ct, hold)
                    dst_ct += 1

            emit_agg(prev)
            while dst_ct < 4:
                dst_stage(pending[0], pending[1], dst_ct, hold)
                dst_ct += 1

    nc.finalize()
    return nc


def _prep_inputs(inputs):
    """Host-side shard/pack. Returns (in_maps, nt_b, node_memory, node_ids)."""
    node_memory = np.ascontiguousarray(np.asarray(inputs["node_memory"], np.float32))
    node_features = np.asarray(inputs["node_features"], np.float32)
    edge_features = np.asarray(inputs["edge_features"], np.float32)
    time_encoding = np.asarray(inputs["time_encoding"], np.float32)
    node_ids = np.asarray(inputs["node_ids"]).astype(np.int64)
    source_ids = np.asarray(inputs["source_ids"]).astype(np.int64)
    edge_ids = np.asarray(inputs["edge_ids"]).astype(np.int64)
    dest_seg = np.asarray(inputs["dest_seg"]).astype(np.int64)
    W_read = np.asarray(inputs["W_read"], np.float32)
    b_read = np.asarray(inputs["b_read"], np.float32)
    W_msg = np.asarray(inputs["W_msg"], np.float32)
    b_msg = np.asarray(inputs["b_msg"], np.float32)
    W_agg = np.asarray(inputs["W_agg"], np.float32)
    b_agg = np.asarray(inputs["b_agg"], np.float32)
    W_upd = np.asarray(inputs["W_upd"], np.float32)
    b_upd = np.asarray(inputs["b_upd"], np.float32)
    W_write = np.asarray(inputs["W_write"], np.float32)
    b_write = np.asarray(inputs["b_write"], np.float32)

    n_edge = dest_seg.shape[0]

    cnt = np.bincount(dest_seg, minlength=N_DEST)
    inv_cnt = np.zeros(N_DEST, np.float32)
    nz = cnt > 0
    inv_cnt[nz] = 1.0 / cnt[nz]

    # 64 global dest blocks of 128; block B's edges are dest_seg in [B*128,(B+1)*128)
    bounds = np.searchsorted(dest_seg, np.arange(0, N_DEST + 1, P))
    per_block = np.diff(bounds)
    nt_b = max(1, math.ceil(per_block.max() / P))
    block_cap = nt_b * P
    e_cap = 8 * block_cap
    NT = 8 * nt_b

    # per-core edge selection (padded); esel indexes into the edge arrays
    esel = np.zeros((N_CORES, e_cap), np.int64)
    valid = np.zeros((N_CORES, e_cap), bool)
    for c in range(N_CORES):
        for blk in range(8):
            B = c * 8 + blk
            lo, hi = int(bounds[B]), int(bounds[B + 1])
            off = blk * block_cap
            esel[c, off:off + hi - lo] = np.arange(lo, hi)
            valid[c, off:off + hi - lo] = True
    esel_f = esel.reshape(-1)
    valid_f = valid.reshape(-1)

    nodecat = np.concatenate([node_memory, node_features], axis=1)  # [N,256]

    # srcTi: per (core, block, chunk): [mem.T | feat.T] interleaved columns
    src_rows = nodecat[source_ids[esel_f]].astype(BF16)    # [8*e_cap, 256]
    src_rows = src_rows.reshape(N_CORES, e_cap, 256)
    srcTi = np.empty((N_CORES, P, 2 * e_cap), BF16)
    for b in range(8):
        coff = 0
        for csz in _chunk_plan(nt_b, b == 0):
            wkd = csz * P
            o = b * block_cap + coff * P
            seg = src_rows[:, o:o + wkd, :]                # [8, w, 256]
            srcTi[:, :, 2 * o:2 * o + wkd] = seg[:, :, :P].transpose(0, 2, 1)
            srcTi[:, :, 2 * o + wkd:2 * o + 2 * wkd] = \
                seg[:, :, P:].transpose(0, 2, 1)
            coff += csz

    ef_rows = edge_features[edge_ids[esel_f]]
    t_rows = time_encoding[np.minimum(esel_f, n_edge - 1)]
    eft = np.concatenate(
        [ef_rows, t_rows, np.ones((len(esel_f), 1), np.float32)], axis=1)
    efts = np.ascontiguousarray(
        eft.reshape(N_CORES, e_cap, 97).transpose(0, 2, 1)).astype(BF16)

    ld_e = (dest_seg[esel_f] % P).astype(np.float32)
    ld_e[~valid_f] = -1.0
    scale_e = inv_cnt[dest_seg[esel_f]].astype(np.float32)
    scale_e[~valid_f] = 0.0
    ld_pack = np.ascontiguousarray(
        ld_e.reshape(N_CORES, NT, P).transpose(0, 2, 1))
    sc_pack = np.ascontiguousarray(
        scale_e.reshape(N_CORES, NT, P).transpose(0, 2, 1))

    iota_h = np.tile(np.arange(P, dtype=np.float32), 4)[None, :].repeat(P, 0)
    iota_h = np.ascontiguousarray(iota_h).astype(BF16)     # [128, 512]

    drows = nodecat[node_ids]                                  # [8192, 256]
    dstT = np.ascontiguousarray(
        drows.reshape(N_CORES, 1024, 256).transpose(0, 2, 1)
    ).astype(BF16).reshape(N_CORES, 2, P, 1024)

    wr_h = np.ascontiguousarray(W_read.reshape(2, P, P)).astype(BF16)
    wm0_h = np.ascontiguousarray(W_msg[:P]).astype(BF16)
    wm1_h = np.ascontiguousarray(
        np.concatenate([W_msg[P:], b_msg[None, :]], axis=0)).astype(BF16)
    wa_h = np.ascontiguousarray(W_agg.reshape(2, P, P)).astype(BF16)
    wu_h = np.ascontiguousarray(W_upd.reshape(2, P, P)).astype(BF16)
    ww_h = np.ascontiguousarray(W_write).astype(BF16)
    br_h = np.ascontiguousarray(b_read[:, None]).astype(np.float32)
    ba_h = np.ascontiguousarray(b_agg[:, None]).astype(np.float32)
    bu_h = np.ascontiguousarray(b_upd[:, None]).astype(np.float32)
    bw_h = np.ascontiguousarray(b_write[:, None]).astype(np.float32)

    in_maps = []
    for c in range(N_CORES):
        in_maps.append({
            "srcTi": srcTi[c], "efts": efts[c],
            "ldest": ld_pack[c], "scale": sc_pack[c], "iota": iota_h,
            "dstT": dstT[c],
            "wr": wr_h, "wm0": wm0_h, "wm1": wm1_h, "wa": wa_h, "wu": wu_h,
            "ww": ww_h, "br": br_h, "ba": ba_h, "bu": bu_h, "bw": bw_h,
        })
    return in_maps, nt_b, node_memory, node_ids


def run(inputs, trace=False, **kw):
    in_maps, nt_b, node_memory, node_ids = _prep_inputs(inputs)
    if nt_b not in _PROG_CACHE:
        _PROG_CACHE[nt_b] = _build_program(nt_b)
    nc = _PROG_CACHE[nt_b]
    res = run_bass_kernel_spmd(nc, in_maps, core_ids=list(range(N_CORES)),
                               trace=trace, **kw)
    wt = np.concatenate(
        [np.asarray(res.results[c]["writeT"], np.float32).T
         for c in range(N_CORES)], axis=0)             # [8192, 128]
    out = node_memory.copy()
    out[node_ids] = wt
    return out, res


def kernel(**inputs) -> np.ndarray:
    out, _ = run(inputs, trace=False)
    return out


# revision 4
# speedup vs baseline: 2.8664x; 2.8664x over previous
"""Trainium2 Bass kernel for LocalDualDirectedMessagePassingLayer.

Strategy (8 cores, dest-sharded):
  - Each core owns 1024 destination segments (8 blocks of 128 dests).
  - dest_seg is sorted, so each dest block's edges are contiguous; host pads
    each block's edge list to NT_B*128 and packs per core:
      srcTi [128, 2*e_cap]  chunk-interleaved (mem|feat per chunk) bf16
      efts  [97, e_cap]     concat(edge_features[edge_ids], time_enc, ones).T
      S_d   [128, e_cap]    fp8 0/1 one-hot (edge-in-tile x (tile,dest))
      invc  [128, 1024]     1/cnt per dest, replicated across partitions
  - Device per 4-tile sup: read MLP (2 matmuls, N=512) + relu on ACT;
    per tile msg MLP (2 matmuls N=128); one DVE relu per sup; agg matmuls
    (lhsT=msgs bf16, rhs=S fp8) software-pipelined one sup behind so the
    PE never stalls on the DVE. 1/cnt is applied when copying the block's
    aggregate out of PSUM (tensor_tensor mult by invc).
  - Per block: dst-side MLP chain (agg/upd/write) -> tanh -> writeT.
  - Host: transpose writeT, scatter rows into a copy of node_memory.
Issue queues: srcTi+out on Sync (hwdge), efts+S on GpSimd (swdge),
constants on Scalar (hwdge).
"""

import sys

sys.path.insert(0, "/opt/trn_rl_repo")

import math

import ml_dtypes
import numpy as np

import concourse.bass as bass
import concourse.mybir as mybir
import concourse.tile as tile
from concourse import bacc
from concourse.bass_utils import run_bass_kernel_spmd

BF16 = ml_dtypes.bfloat16
FP8 = ml_dtypes.float8_e4m3
N_CORES = 8
P = 128
N_DEST = 8192
D_MEM = 128

_PROG_CACHE: dict[int, object] = {}


def _chunk_plan(nt_b: int, first_block: bool):
    """DMA chunks in tiles. One chunk per block, except block 0 splits the
    first 4 tiles out so the PE starts early."""
    if first_block and nt_b > 4:
        return [4, nt_b - 4]
    return [nt_b]


def _sup_plan(nt_b: int):
    """4-tile super-tiles, with one tail sup of nt_b%4 tiles."""
    sups = [4] * (nt_b // 4)
    if nt_b % 4:
        sups.append(nt_b % 4)
    return sups


def _build_program(nt_b: int):
    NT = 8 * nt_b
    e_cap = NT * P
    bcap = nt_b * P

    nc = bacc.Bacc("TRN2", target_bir_lowering=False, debug=False,
                   num_devices=N_CORES)
    f32 = mybir.dt.float32
    bf16 = mybir.dt.bfloat16
    fp8 = mybir.dt.float8e4
    AF = mybir.ActivationFunctionType
    OP = mybir.AluOpType

    srcTi = nc.dram_tensor("srcTi", [P, 2 * e_cap], bf16, kind="ExternalInput")
    efts = nc.dram_tensor("efts", [97, e_cap], bf16, kind="ExternalInput")
    S_d = nc.dram_tensor("S_d", [P, e_cap], fp8, kind="ExternalInput")
    invc = nc.dram_tensor("invc", [P, 1024], bf16, kind="ExternalInput")
    dstT = nc.dram_tensor("dstT", [2, P, 1024], bf16, kind="ExternalInput")
    wr = nc.dram_tensor("wr", [2, P, P], bf16, kind="ExternalInput")
    wm0 = nc.dram_tensor("wm0", [P, P], bf16, kind="ExternalInput")
    wm1 = nc.dram_tensor("wm1", [97, P], bf16, kind="ExternalInput")
    wa = nc.dram_tensor("wa", [2, P, P], bf16, kind="ExternalInput")
    wu = nc.dram_tensor("wu", [2, P, P], bf16, kind="ExternalInput")
    ww = nc.dram_tensor("ww", [P, P], bf16, kind="ExternalInput")
    br = nc.dram_tensor("br", [P, 1], f32, kind="ExternalInput")
    ba = nc.dram_tensor("ba", [P, 1], f32, kind="ExternalInput")
    bu = nc.dram_tensor("bu", [P, 1], f32, kind="ExternalInput")
    bw = nc.dram_tensor("bw", [P, 1], f32, kind="ExternalInput")
    out_d = nc.dram_tensor("writeT", [P, 1024], f32, kind="ExternalOutput")

    with tile.TileContext(nc) as tc:
        with (
            tc.tile_pool(name="const", bufs=1) as cp,
            tc.tile_pool(name="io", bufs=3) as iop,
            tc.tile_pool(name="mid", bufs=8) as midp,
            tc.tile_pool(name="rdps", bufs=2, space="PSUM") as rdps,
            tc.tile_pool(name="mgps", bufs=2, space="PSUM") as mgps,
            tc.tile_pool(name="aggps", bufs=2, space="PSUM") as aggps,
            tc.tile_pool(name="dstps", bufs=1, space="PSUM") as dstps,
        ):
            def cload(ap, shape, dtype, tag):
                t = cp.tile(shape, dtype, tag=tag)
                nc.scalar.dma_start(out=t[:], in_=ap)
                return t

            # critical-path constants first (PE read weights)
            wr0 = cload(wr[0, :, :], [P, P], bf16, "wr0")
            wr1 = cload(wr[1, :, :], [P, P], bf16, "wr1")
            br_t = cload(br[:, :], [P, 1], f32, "br")
            wm0_t = cload(wm0[:, :], [P, P], bf16, "wm0")
            wm1_t = cload(wm1[:, :], [97, P], bf16, "wm1")
            invc_t = cload(invc[:, :], [P, 1024], bf16, "invc")
            dstT0 = cload(dstT[0, :, :], [P, 1024], bf16, "dstT0")
            dstT1 = cload(dstT[1, :, :], [P, 1024], bf16, "dstT1")
            wa0 = cload(wa[0, :, :], [P, P], bf16, "wa0")
            wa1 = cload(wa[1, :, :], [P, P], bf16, "wa1")
            wu0 = cload(wu[0, :, :], [P, P], bf16, "wu0")
            wu1 = cload(wu[1, :, :], [P, P], bf16, "wu1")
            ww_t = cload(ww[:, :], [P, P], bf16, "ww")
            ba_t = cload(ba[:, :], [P, 1], f32, "ba")
            bu_t = cload(bu[:, :], [P, 1], f32, "bu")
            bw_t = cload(bw[:, :], [P, 1], f32, "bw")

            def dst_stage(b, agg_ps, stage, hold):
                dc = slice(b * P, (b + 1) * P)
                if stage == 0:
                    mmean = midp.tile([P, P], bf16, tag="mmean")
                    nc.vector.tensor_mul(mmean[:], agg_ps[:], invc_t[:, dc])
                    drp = dstps.tile([P, P], f32, tag="dst")
                    nc.tensor.matmul(drp[:], lhsT=wr0[:], rhs=dstT0[:, dc],
                                     start=True, stop=False)
                    nc.tensor.matmul(drp[:], lhsT=wr1[:], rhs=dstT1[:, dc],
                                     start=False, stop=True)
                    dstr = midp.tile([P, P], bf16, tag="dstr")
                    nc.scalar.activation(dstr[:], drp[:], AF.Relu, bias=br_t[:, :1])
                    hold.update(mmean=mmean, dstr=dstr)
                elif stage == 1:
                    agp = dstps.tile([P, P], f32, tag="dst")
                    nc.tensor.matmul(agp[:], lhsT=wa0[:], rhs=hold["dstr"][:],
                                     start=True, stop=False)
                    nc.tensor.matmul(agp[:], lhsT=wa1[:], rhs=hold["mmean"][:],
                                     start=False, stop=True)
                    aggT = midp.tile([P, P], bf16, tag="aggT")
                    nc.scalar.activation(aggT[:], agp[:], AF.Relu, bias=ba_t[:, :1])
                    hold.update(aggT=aggT)
                elif stage == 2:
                    upp = dstps.tile([P, P], f32, tag="dst")
                    nc.tensor.matmul(upp[:], lhsT=wu0[:], rhs=hold["aggT"][:],
                                     start=True, stop=False)
                    nc.tensor.matmul(upp[:], lhsT=wu1[:], rhs=hold["dstr"][:],
                                     start=False, stop=True)
                    updT = midp.tile([P, P], bf16, tag="updT")
                    nc.scalar.activation(updT[:], upp[:], AF.Relu, bias=bu_t[:, :1])
                    hold.update(updT=updT)
                else:
                    wrp = dstps.tile([P, P], f32, tag="dst")
                    nc.tensor.matmul(wrp[:], lhsT=ww_t[:], rhs=hold["updT"][:],
                                     start=True, stop=True)
                    wout = midp.tile([P, P], f32, tag="wout")
                    nc.scalar.activation(wout[:], wrp[:], AF.Tanh, bias=bw_t[:, :1])
                    nc.sync.dma_start(out=out_d[:, dc], in_=wout[:])

            sups = _sup_plan(nt_b)

            # global sup list: (block, tile0_global, ntiles, new_chunk_info)
            sup_list = []
            for b in range(8):
                chunks = _chunk_plan(nt_b, b == 0)
                coff, chunk_bounds = 0, []
                for csz in chunks:
                    chunk_bounds.append((coff, csz))
                    coff += csz
                soff, ci = 0, 0
                for ntile in sups:
                    newc = None
                    if ci < len(chunk_bounds) and chunk_bounds[ci][0] == soff:
                        newc = chunk_bounds[ci]
                        ci += 1
                    sup_list.append((b, b * nt_b + soff, ntile, newc))
                    soff += ntile

            prev = None          # info for sup s-1 awaiting agg matmuls
            pending = None       # (block, agg_ps) awaiting dst stages
            hold = {}
            dst_ct = 4
            agg_ps = aggps.tile([P, P], f32, tag="agg")

            def emit_agg(pv):
                nonlocal pending, agg_ps, dst_ct, hold
                (pb, ptile0, pnt, pmsgs, pS, pj) = pv
                t_in_b = ptile0 - pb * nt_b
                for q in range(pnt):
                    qs = slice(q * P, (q + 1) * P)
                    Ss = slice(pj + q * P, pj + (q + 1) * P)
                    nc.tensor.matmul(agg_ps[:], lhsT=pmsgs[:, qs],
                                     rhs=pS[:, Ss],
                                     start=(t_in_b + q == 0),
                                     stop=(t_in_b + q == nt_b - 1))
                if t_in_b + pnt == nt_b:
                    pending = (pb, agg_ps)
                    dst_ct = 0
                    hold = {}
                    agg_ps = aggps.tile([P, P], f32, tag="agg")

            cur_chunk = None
            for s, (b, tile0, ntile, newc) in enumerate(sup_list):
                if newc is not None:
                    c_t0, c_sz = newc
                    w = c_sz * P
                    o = (b * nt_b + c_t0) * P
                    ct = iop.tile([P, 2 * bcap], bf16, tag="srcT")
                    nc.sync.dma_start(out=ct[:, :2 * w],
                                      in_=srcTi[:, 2 * o:2 * o + 2 * w])
                    ef = iop.tile([97, bcap], bf16, tag="ef")
                    nc.gpsimd.dma_start(out=ef[:, :w], in_=efts[:, o:o + w])
                    St = iop.tile([P, bcap], fp8, tag="S")
                    nc.gpsimd.dma_start(out=St[:, :w], in_=S_d[:, o:o + w])
                    cur_chunk = (ct, ef, St, w, c_t0)

                ct, ef, St, w, c_t0 = cur_chunk
                j = (tile0 - b * nt_b - c_t0) * P   # col offset within chunk
                W = ntile * P
                s0 = ct[:, j:j + W]
                s1 = ct[:, w + j:w + j + W]
                efs = ef[:, j:j + W]

                # read MLP
                rd = rdps.tile([P, 512], f32, tag="rd")
                nc.tensor.matmul(rd[:, :W], lhsT=wr0[:], rhs=s0,
                                 start=True, stop=False)
                nc.tensor.matmul(rd[:, :W], lhsT=wr1[:], rhs=s1,
                                 start=False, stop=True)
                srT = midp.tile([P, 512], bf16, tag="srT")
                nc.scalar.activation(srT[:, :W], rd[:, :W], AF.Relu,
                                     bias=br_t[:, :1])

                # msg MLP per tile
                mg4 = mgps.tile([P, 512], f32, tag="mg")
                for q in range(ntile):
                    qs = slice(q * P, (q + 1) * P)
                    nc.tensor.matmul(mg4[:, qs], lhsT=srT[:, qs], rhs=wm0_t[:],
                                     start=True, stop=False)
                    nc.tensor.matmul(mg4[:, qs], lhsT=efs[:, qs], rhs=wm1_t[:],
                                     start=False, stop=True)

                # agg matmuls for previous sup (msgs ready by now)
                if prev is not None:
                    emit_agg(prev)

                msgs4 = midp.tile([P, 512], bf16, tag="msgs")
                nc.vector.tensor_scalar_max(msgs4[:, :W], mg4[:, :W], 0.0)
                prev = (b, tile0, ntile, msgs4, St, j)

                if pending is not None and dst_ct < 4:
                    dst_stage(pending[0], pending[1], dst_ct, hold)
                    dst_ct += 1

            emit_agg(prev)
            while dst_ct < 4:
                dst_stage(pending[0], pending[1], dst_ct, hold)
                dst_ct += 1

    nc.finalize()
    return nc


def _prep_inputs(inputs):
    """Host-side shard/pack. Returns (in_maps, nt_b, node_memory, node_ids)."""
    node_memory = np.ascontiguousarray(np.asarray(inputs["node_memory"], np.float32))
    node_features = np.asarray(inputs["node_features"], np.float32)
    edge_features = np.asarray(inputs["edge_features"], np.float32)
    time_encoding = np.asarray(inputs["time_encoding"], np.float32)
    node_ids = np.asarray(inputs["node_ids"]).astype(np.int64)
    source_ids = np.asarray(inputs["source_ids"]).astype(np.int64)
    edge_ids = np.asarray(inputs["edge_ids"]).astype(np.int64)
    dest_seg = np.asarray(inputs["dest_seg"]).astype(np.int64)
    W_read = np.asarray(inputs["W_read"], np.float32)
    b_read = np.asarray(inputs["b_read"], np.float32)
    W_msg = np.asarray(inputs["W_msg"], np.float32)
    b_msg = np.asarray(inputs["b_msg"], np.float32)
    W_agg = np.asarray(inputs["W_agg"], np.float32)
    b_agg = np.asarray(inputs["b_agg"], np.float32)
    W_upd = np.asarray(inputs["W_upd"], np.float32)
    b_upd = np.asarray(inputs["b_upd"], np.float32)
    W_write = np.asarray(inputs["W_write"], np.float32)
    b_write = np.asarray(inputs["b_write"], np.float32)

    n_edge = dest_seg.shape[0]

    cnt = np.bincount(dest_seg, minlength=N_DEST)
    inv_cnt = np.zeros(N_DEST, np.float32)
    nz = cnt > 0
    inv_cnt[nz] = 1.0 / cnt[nz]

    # 64 global dest blocks of 128; block B's edges are dest_seg in [B*128,(B+1)*128)
    bounds = np.searchsorted(dest_seg, np.arange(0, N_DEST + 1, P))
    per_block = np.diff(bounds)
    nt_b = max(1, math.ceil(per_block.max() / P))
    block_cap = nt_b * P
    e_cap = 8 * block_cap
    NT = 8 * nt_b

    # per-core edge selection (padded); esel indexes into the edge arrays
    esel = np.zeros((N_CORES, e_cap), np.int64)
    valid = np.zeros((N_CORES, e_cap), bool)
    for c in range(N_CORES):
        for blk in range(8):
            B = c * 8 + blk
            lo, hi = int(bounds[B]), int(bounds[B + 1])
            off = blk * block_cap
            esel[c, off:off + hi - lo] = np.arange(lo, hi)
            valid[c, off:off + hi - lo] = True
    esel_f = esel.reshape(-1)
    valid_f = valid.reshape(-1)

    nodecat = np.concatenate([node_memory, node_features], axis=1)  # [N,256]

    # srcTi: per (core, block, chunk): [mem.T | feat.T] columns
    src_rows = nodecat[source_ids[esel_f]].astype(BF16)    # [8*e_cap, 256]
    src_rows = src_rows.reshape(N_CORES, e_cap, 256)
    srcTi = np.empty((N_CORES, P, 2 * e_cap), BF16)
    for b in range(8):
        coff = 0
        for csz in _chunk_plan(nt_b, b == 0):
            wkd = csz * P
            o = b * block_cap + coff * P
            seg = src_rows[:, o:o + wkd, :]                # [8, w, 256]
            srcTi[:, :, 2 * o:2 * o + wkd] = seg[:, :, :P].transpose(0, 2, 1)
            srcTi[:, :, 2 * o + wkd:2 * o + 2 * wkd] = \
                seg[:, :, P:].transpose(0, 2, 1)
            coff += csz

    ef_rows = edge_features[edge_ids[esel_f]]
    t_rows = time_encoding[np.minimum(esel_f, n_edge - 1)]
    eft = np.concatenate(
        [ef_rows, t_rows, np.ones((len(esel_f), 1), np.float32)], axis=1)
    efts = np.ascontiguousarray(
        eft.reshape(N_CORES, e_cap, 97).transpose(0, 2, 1)).astype(BF16)

    # one-hot S (0/1, fp8): S[c][e_in_tile, t*128 + d] = 1 iff edge's ldest==d
    ld_e = dest_seg[esel_f] % P
    ld_e[~valid_f] = 0
    S_flat = np.zeros((N_CORES * e_cap, P), np.float32)
    S_flat[np.arange(N_CORES * e_cap), ld_e] = 1.0
    S_flat[~valid_f] = 0.0
    S_pack = np.ascontiguousarray(
        S_flat.reshape(N_CORES, NT, P, P).transpose(0, 2, 1, 3)
        .reshape(N_CORES, P, e_cap)).astype(FP8)

    invc_h = np.ascontiguousarray(
        np.broadcast_to(inv_cnt.reshape(N_CORES, 1, 1024),
                        (N_CORES, P, 1024))).astype(BF16)

    drows = nodecat[node_ids]                                  # [8192, 256]
    dstT = np.ascontiguousarray(
        drows.reshape(N_CORES, 1024, 256).transpose(0, 2, 1)
    ).astype(BF16).reshape(N_CORES, 2, P, 1024)

    wr_h = np.ascontiguousarray(W_read.reshape(2, P, P)).astype(BF16)
    wm0_h = np.ascontiguousarray(W_msg[:P]).astype(BF16)
    wm1_h = np.ascontiguousarray(
        np.concatenate([W_msg[P:], b_msg[None, :]], axis=0)).astype(BF16)
    wa_h = np.ascontiguousarray(W_agg.reshape(2, P, P)).astype(BF16)
    wu_h = np.ascontiguousarray(W_upd.reshape(2, P, P)).astype(BF16)
    ww_h = np.ascontiguousarray(W_write).astype(BF16)
    br_h = np.ascontiguousarray(b_read[:, None]).astype(np.float32)
    ba_h = np.ascontiguousarray(b_agg[:, None]).astype(np.float32)
    bu_h = np.ascontiguousarray(b_upd[:, None]).astype(np.float32)
    bw_h = np.ascontiguousarray(b_write[:, None]).astype(np.float32)

    in_maps = []
    for c in range(N_CORES):
        in_maps.append({
            "srcTi": srcTi[c], "efts": efts[c], "S_d": S_pack[c],
            "invc": invc_h[c], "dstT": dstT[c],
            "wr": wr_h, "wm0": wm0_h, "wm1": wm1_h, "wa": wa_h, "wu": wu_h,
            "ww": ww_h, "br": br_h, "ba": ba_h, "bu": bu_h, "bw": bw_h,
        })
    return in_maps, nt_b, node_memory, node_ids


def run(inputs, trace=False, **kw):
    in_maps, nt_b, node_memory, node_ids = _prep_inputs(inputs)
    if nt_b not in _PROG_CACHE:
        _PROG_CACHE[nt_b] = _build_program(nt_b)
    nc = _PROG_CACHE[nt_b]
    res = run_bass_kernel_spmd(nc, in_maps, core_ids=list(range(N_CORES)),
                               trace=trace, **kw)
    wt = np.concatenate(
        [np.asarray(res.results[c]["writeT"], np.float32).T
         for c in range(N_CORES)], axis=0)             # [8192, 128]
    out = node_memory.copy()
    out[node_ids] = wt
    return out, res


def kernel(**inputs) -> np.ndarray:
    out, _ = run(inputs, trace=False)
    return out
